# revision 16
# baseline (speedup 1.0000x reference)
"""Trainium2 Bass kernel for nn_Head (single-head causal attention).

Contract: kernel(**inputs) takes FULL inputs (x [8,2048,1024] f32,
Wk/Wq/Wv [64,1024] f32) and returns the FULL output [8,2048,64] f32.
Data-parallel over batch B=8 across the 8 NeuronCores (one batch row per
core); each core runs an identical single-core program.

v5.1 design (descending-chunk streaming; engineered against v4/v5 traces):
  * x streams in four 512-column t-chunks in REVERSE order. ST(i) needs
    kt[:, 128i:2048] and qt tile i, so descending i lets attention on
    chunk c start while earlier chunks still DMA. Projections for chunk
    c-1 are SPREAD through chunk c's attention (4 matmuls per i) so the
    PE never idles during exp waits and the HAM clock stays at 2.4GHz.
  * PSUM: stp = 2 rotating [128,1024]f32 slots (tag "st") for ST tiles +
    transient claims (chunk-3 proj, v0, v-transpose staging, epilogue
    broadcast). otp = 4 persistent PV banks. Bank 0's PSUM hosts the
    kq projections of chunks 2/1/0 before PV first touches it at i=3;
    bank 1 hosts v of chunks 2/1 (first PV touch at i=7). That keeps
    the st rotation free for the ST<->exp ping-pong.
  * PV bank j accumulates i=4j+3 down to 0. Its first matmul is forced
    full width (below-diagonal pt cols zeroed by gpsimd memset) so
    start=True clears the whole bank.
  * ACT does ONLY Exp (table preloaded in the DMA lead-in; evacuation is
    all DVE so there are no 1.28us activation-table switches).
  * v5's tail was 10us of 16 small out-DMAs (650ns DGE delay each) after
    per-128 transposes. v5.1 epilogue is transpose-free: per bank, the
    denominator row is cast bf16, broadcast across partitions with a K=1
    ones-matmul, one DVE tensor_tensor divide -> o2 [64, T] bf16, then
    TWO batched out DMAs. Host transposes [H,T] -> [T,H] and casts f32
    (marshaling only).
"""

import sys

if "/opt/trn_rl_repo" not in sys.path:
    sys.path.insert(0, "/opt/trn_rl_repo")

import numpy as np

B = 8
T = 2048
C = 1024
H = 64
P = 128
CB = C // P        # 8 contraction chunks
TJ = T // 512      # 4 column chunks of 512
NT = T // P        # 16 s-tiles
N_CORES = 8

_NC_CACHE = {}


def _build_nc():
    import concourse.bass as bass
    import concourse.mybir as mybir
    import concourse.tile as tile
    from concourse.bass import ts
    from concourse.masks import make_identity

    fp32 = mybir.dt.float32
    bf16 = mybir.dt.bfloat16
    EXP = mybir.ActivationFunctionType.Exp

    nc = bass.Bass(target_bir_lowering=False, debug=False)
    xt_d = nc.declare_dram_parameter("xt", [C, T], bf16, isOutput=False)
    wkq_d = nc.declare_dram_parameter("wkq", [C, P], bf16, isOutput=False)
    wv_d = nc.declare_dram_parameter("wv", [C, H], bf16, isOutput=False)
    out_d = nc.declare_dram_parameter("out", [T, H], bf16, isOutput=True)

    from contextlib import ExitStack

    with tile.TileContext(nc) as tc, ExitStack() as stk:
        pers = stk.enter_context(tc.tile_pool(name="pers", bufs=1))
        xt_sb = pers.tile([P, CB, T], bf16, tag="xt_sb", name="xt_sb")
        wkq_sb = pers.tile([P, CB, P], bf16, tag="wkq_sb", name="wkq_sb")
        wv_sb = pers.tile([P, CB, H], bf16, tag="wv_sb", name="wv_sb")
        kt_sb = pers.tile([H, T], bf16, tag="kt_sb", name="kt_sb")
        qt_sb = pers.tile([H, T], bf16, tag="qt_sb", name="qt_sb")
        vt_sb = pers.tile([H, TJ, 512], bf16, tag="vt_sb", name="vt_sb")
        vaug_sb = pers.tile([P, NT, H + 1], bf16, tag="vaug_sb", name="vaug_sb")
        ot_sb = pers.tile([H + 1, T], fp32, tag="ot_sb", name="ot_sb")
        o_sb = pers.tile([P, NT, H], bf16, tag="o_sb", name="o_sb")
        rec_sb = pers.tile([P, NT], fp32, tag="rec_sb", name="rec_sb")
        identf = pers.tile([H + 1, H + 1], fp32, tag="identf", name="identf")
        identb = pers.tile([H, H], bf16, tag="identb", name="identb")
        dummy_sb = pers.tile([P, 512], bf16, tag="dummy_sb", name="dummy_sb")
        tl_sb = pers.tile([1, 8], fp32, tag="tl_sb", name="tl_sb")

        # ---- constants FIRST: they live on gpsimd, which also software-
        # generates its DMA descriptors (SWDGE) — emitted after the DMAs
        # they'd stall the warm-up dummies ~10us. Exp table preloads on ACT. ----
        make_identity(nc, identb[:])
        make_identity(nc, identf[:])
        nc.gpsimd.memset(dummy_sb[:], 0.0)
        nc.gpsimd.memset(tl_sb[:], 0.0)
        nc.any.memset(vaug_sb[:, :, H], 1.0)
        nc.scalar.activation(tl_sb[:], tl_sb[:], EXP)

        # ---- input DMAs across the three DGE queues (sync/scalar HWDGE +
        # gpsimd SWDGE), weights first, x t-chunks newest-needed first; each
        # queue carries a cb stripe so projection cb 0..7 can start as
        # stripes land. gpsimd carries only x (its SWDGE descriptor
        # generation must clear before the diagonal masks need the engine). ----
        wkq_r = wkq_d.rearrange("(o p) m -> p o m", p=P)
        xt_r = xt_d.rearrange("(o p) m -> p o m", p=P)
        nc.sync.dma_start(wkq_sb[:, 0:4, :], wkq_r[:, 0:4, :])
        nc.scalar.dma_start(wkq_sb[:, 4:8, :], wkq_r[:, 4:8, :])
        nc.scalar.dma_start(wv_sb[:], wv_d.rearrange("(o p) m -> p o m", p=P))
        engs = [nc.sync, nc.scalar, nc.gpsimd]
        stripes = [(0, 3), (3, 6), (6, 8)]
        for c in [3, 2, 1, 0]:
            for q, (a, b) in enumerate(stripes):
                engs[q].dma_start(
                    xt_sb[:, a:b, ts(c, 512)], xt_r[:, a:b, ts(c, 512)]
                )

        # ---- HAM warm-up: dummy matmuls bridge the DMA lead-in so real
        # matmuls run at 2.4GHz from the start ----
        with tc.tile_pool(name="warm", bufs=1, space="PSUM") as wp:
            wps = wp.tile([H, P], fp32, tag="w", name="warm")
            for _ in range(34):
                nc.tensor.matmul(
                    wps, identb[:], dummy_sb[0:H, 0:P], start=True, stop=True
                )

        with (
            tc.tile_pool(name="stp", bufs=2, space="PSUM") as stp,
            tc.tile_pool(name="otp", bufs=4, space="PSUM") as otp,
            tc.tile_pool(name="ptp", bufs=6) as ptp,
        ):
            # full [128,512] handles: rows 0:65 are the PV accumulators;
            # bank 0 / bank 1 host late-chunk kq / v projections first
            ot_full = [otp.tile([P, 512], fp32, tag="ot", name=f"otf{j}") for j in range(TJ)]
            ot_ps = [otf[0 : H + 1, :] for otf in ot_full]

            def emit_st(i):
                j0 = i // 4
                pts = {}
                for jj2 in range(i // 8, 2):
                    st = stp.tile([P, 1024], fp32, tag="st", name=f"st{i}_{jj2}")
                    pt = ptp.tile([P, 1024], bf16, tag="pt", name=f"pt{i}_{jj2}")
                    estart = None
                    for hh in range(2):
                        j = 2 * jj2 + hh
                        if j < j0:
                            continue
                        o = max(0, 128 * i - 512 * j)
                        lo = 512 * hh + o
                        nc.tensor.matmul(
                            st[:, lo : 512 * (hh + 1)], qt_sb[:, ts(i, P)],
                            kt_sb[:, 512 * j + o : 512 * (j + 1)],
                            start=True, stop=True,
                        )
                        if estart is None:
                            estart = lo
                    nc.scalar.activation(pt[:, estart:1024], st[:, estart:1024], EXP)
                    if jj2 == i // 8:
                        # causal mask of the diagonal 128x128 block:
                        # keep pt[s, t] where t - s >= 0, else 0
                        dlo = 128 * (i % 8)
                        nc.gpsimd.affine_select(
                            out=pt[:, dlo : dlo + P],
                            in_=pt[:, dlo : dlo + P],
                            pattern=[[1, P]],
                            compare_op=mybir.AluOpType.is_ge,
                            fill=0.0,
                            base=0,
                            channel_multiplier=-1,
                        )
                    if i % 4 == 3 and jj2 == i // 8:
                        # this i opens PV bank j0: zero the below-diagonal
                        # cols of the piece so the bank's first PV matmul can
                        # be full width (start=True then clears the whole bank)
                        zlo = 512 * (j0 % 2)
                        nc.gpsimd.memset(pt[:, zlo : zlo + 384], 0.0)
                    pts[jj2] = pt
                return pts

            def emit_pv(i, pts):
                j0 = i // 4
                for j in range(j0, TJ):
                    o = 0 if i == 4 * j + 3 else max(0, 128 * i - 512 * j)
                    pt = pts[j // 2]
                    lo = 512 * (j % 2) + o
                    nc.tensor.matmul(
                        ot_ps[j][:, o:512], vaug_sb[:, i, :],
                        pt[:, lo : 512 * (j % 2) + 512],
                        start=(i == 4 * j + 3), stop=(i == 0),
                    )

            def proj_mms(target, w_sb, c, cbs):
                for cb in cbs:
                    nc.tensor.matmul(
                        target, w_sb[:, cb, :], xt_sb[:, cb, ts(c, 512)],
                        start=(cb == 0), stop=(cb == CB - 1),
                    )

            # chunk-3 projections run in the lead-in from stp slots
            kq3 = stp.tile([P, 512], fp32, tag="st", name="kq3")
            v3 = stp.tile([H, 512], fp32, tag="st", name="v3")
            v0 = None  # claimed at the chunk-0 boundary

            def kq_of(c):
                return kq3 if c == 3 else ot_full[0]

            def v_of(c):
                if c == 3:
                    return v3[0:H, :]
                if c == 0:
                    return v0[0:H, :]
                return ot_full[1][0:H, :]

            # proj work of chunk c-1 spread through chunk c's i-loop:
            # (target, weights, chunk, cb-list) per i-iteration
            def spread_plan(c):
                if c == 3:  # project chunk 2 into ot banks 0/1
                    return [("kq", 2, range(0, 4)), ("kq", 2, range(4, 8)),
                            ("v", 2, range(0, 4)), ("v", 2, range(4, 8))]
                if c == 2:  # project chunk 1
                    return [("kq", 1, range(0, 4)), ("kq", 1, range(4, 8)),
                            ("v", 1, range(0, 4)), ("v", 1, range(4, 8))]
                if c == 1:  # project chunk 0's kq only (v0 needs an stp slot)
                    return [("kq", 0, range(0, 2)), ("kq", 0, range(2, 4)),
                            ("kq", 0, range(4, 6)), ("kq", 0, range(6, 8))]
                return [None, None, None, None]

            proj_mms(kq3, wkq_sb, 3, range(CB))
            proj_mms(v3[0:H, :], wv_sb, 3, range(CB))

            prev = None
            for c in [3, 2, 1, 0]:
                if c == 0:
                    v0 = stp.tile([H, 512], fp32, tag="st", name="v0")
                    proj_mms(v0[0:H, :], wv_sb, 0, range(CB))
                # evacuate chunk c: k rows 0:64 -> kt, q rows 64:128 -> qt
                # (DVE copies can shift partition base), v -> vt
                kq = kq_of(c)
                nc.vector.tensor_copy(kt_sb[:, ts(c, 512)], kq[0:H, :])
                nc.vector.tensor_copy(qt_sb[:, ts(c, 512)], kq[H:P, :])
                nc.vector.tensor_copy(vt_sb[:, c, :], v_of(c))
                # v natural layout: 4 PE transposes + one wide DVE copy
                vg = stp.tile([P, 4, H], bf16, tag="st", name=f"vg{c}")
                for t4 in range(4):
                    nc.tensor.transpose(
                        vg[:, t4, :], vt_sb[:, c, ts(t4, P)], identb[:]
                    )
                nc.vector.tensor_copy(vaug_sb[:, 4 * c : 4 * c + 4, 0:H], vg)
                # attention for this chunk's s-tiles, descending; PV lags ST
                # by one i; next chunk's proj matmuls fill the exp-wait gaps
                plan = spread_plan(c)
                for n, i in enumerate(range(4 * c + 3, 4 * c - 1, -1)):
                    pts = emit_st(i)
                    if prev is not None:
                        emit_pv(prev[0], prev[1])
                    step = plan[n]
                    if step is not None:
                        kind, pc, cbs = step
                        if kind == "kq":
                            proj_mms(ot_full[0], wkq_sb, pc, cbs)
                        else:
                            proj_mms(ot_full[1][0:H, :], wv_sb, pc, cbs)
                    prev = (i, pts)
            emit_pv(prev[0], prev[1])

            # ---- epilogue: banks close together at PV(0). Bank copies go on
            # ACT (one Exp->Copy table switch; exps are done) in parallel
            # with DVE; per 128-tile: PE transpose into a freed stp slot,
            # cheap per-partition reciprocal [128,1], per-partition scale.
            # Output leaves as TWO batched DMAs (v5's 16 small DMAs cost
            # 650ns DGE delay each) ----
            COPY = mybir.ActivationFunctionType.Copy
            out_r = out_d.rearrange("(i p) d -> p i d", p=P)
            for j in range(TJ):
                nc.scalar.activation(ot_sb[:, ts(j, 512)], ot_ps[j], COPY)
                for ii in range(4 * j, 4 * j + 4):
                    ops = stp.tile([P, H + 1], fp32, tag="st", name=f"or{ii}")
                    nc.tensor.transpose(ops, ot_sb[:, ts(ii, P)], identf[:])
                    nc.vector.reciprocal(rec_sb[:, ii : ii + 1], ops[:, H : H + 1])
                    nc.vector.tensor_scalar_mul(
                        o_sb[:, ii, :], ops[:, 0:H], rec_sb[:, ii : ii + 1]
                    )
            nc.sync.dma_start(out_r[:, 0:8, :], o_sb[:, 0:8, :])
            nc.scalar.dma_start(out_r[:, 8:16, :], o_sb[:, 8:16, :])

    return nc


def _split_multiwaits(nc):
    """Walrus codegen only supports one sync-wait command per instruction;
    hoist extra waits onto NoOps inserted just before (same engine queue,
    identical semantics since engines execute their queue in order)."""
    import concourse.mybir as mybir

    n = 0
    for fn in nc.m.functions:
        for block in fn.blocks:
            new_insts = []
            for inst in block.instructions:
                si = inst.sync_info
                if si is not None and si.on_wait and len(si.on_wait) > 1:
                    waits = list(si.on_wait)
                    for w in waits[:-1]:
                        n += 1
                        new_insts.append(
                            mybir.InstNoOp(
                                name=f"WH-{n}", engine=inst.engine, ins=[], outs=[],
                                sync_info=mybir.SyncInfo(on_wait=[w], on_update=[]),
                            )
                        )
                    si.on_wait = waits[-1:]
                new_insts.append(inst)
            block.instructions = new_insts
    return nc


def _get_nc():
    if "nc" not in _NC_CACHE:
        _NC_CACHE["nc"] = _split_multiwaits(_build_nc())
    return _NC_CACHE["nc"]


def _make_in_maps(x, Wk, Wq, Wv):
    import ml_dtypes

    bf16 = ml_dtypes.bfloat16
    scale = 1.0 / np.sqrt(np.float32(C))
    wkq = np.ascontiguousarray(
        np.concatenate([Wk * scale, Wq], axis=0).T.astype(bf16)
    )  # [C, 128]
    wv = np.ascontiguousarray(Wv.T.astype(bf16))  # [C, 64]
    in_maps = []
    for b in range(B):
        xt = np.ascontiguousarray(x[b].T.astype(bf16))  # [C, T]
        in_maps.append({"xt": xt, "wkq": wkq, "wv": wv})
    return in_maps


def run(x, Wk, Wq, Wv, trace=False):
    from concourse.bass_utils import run_bass_kernel_spmd

    nc = _get_nc()
    in_maps = _make_in_maps(x, Wk, Wq, Wv)
    res = run_bass_kernel_spmd(nc, in_maps, core_ids=list(range(N_CORES)), trace=trace)
    # device output is [T, H] bf16; cast to f32 is host-side marshaling
    out = np.stack(
        [np.asarray(res.results[b]["out"]).astype(np.float32) for b in range(B)],
        axis=0,
    )
    return np.ascontiguousarray(out), res


def kernel(x, Wk, Wq, Wv):
    out, _ = run(x, Wk, Wq, Wv, trace=False)
    return out
